# revision 1
# baseline (speedup 1.0000x reference)
"""DGSSM loss (chamfer + coeff MSE + affine MSE) on 8 Trainium2 NeuronCores.

Sharding: data-parallel over batch B=16 -> 2 batches per core.
Per batch on device:
  - affine-transform target cloud via PE matmul (homogeneous coords)
  - d2[n,m] = |p_n|^2 + |t'_m|^2 - 2 p_n.t'_m via one K=9 augmented fp16 PE
    matmul per [128 x 512] tile (fp32 PSUM accumulate; ~1e-4 rel error on
    the loss, verified against an exact numpy model)
  - ACT converts d2 tiles to bf16 SBUF with fused Relu (monotone, so mins
    commute); DVE does row mins (pairwise tensor_tensor(min) folds + final
    reduce) and col-min accumulation; PE transposes the col accumulator for
    the cross-partition min
  - partial sums reduced to scalars via ones-matmuls, DMA'd out
Host: builds rotation matrices (16 tiny 3x3), transposes/packs inputs
(layout only), sums the 8 cores' partial scalars.
"""

import numpy as np

B, N, M = 16, 4096, 4096
KW = 128          # weights dim
NCORES = 8
BPC = B // NCORES  # batches per core
NT = N // 128      # 32 n-tiles
MC = 512           # m chunk (one PSUM bank of fp32)
NMC = M // MC      # 8 m-chunks

W_POINT, W_COEFF, W_AFFINE = 1.0, 0.5, 0.5

_cache = {}


def _build_nc():
    import concourse.bacc as bacc
    import concourse.tile as tile
    import concourse.mybir as mybir

    f32 = mybir.dt.float32
    f16 = mybir.dt.float16
    bf16 = mybir.dt.bfloat16
    Alu = mybir.AluOpType
    Act = mybir.ActivationFunctionType
    Ax = mybir.AxisListType

    nc = bacc.Bacc()

    pred_aug_d = nc.declare_dram_parameter("pred_aug", [BPC, 3, N], f32, isOutput=False)
    # cols 0:M = [targ coords; ones], cols M:M+3 = transform matrix
    targ_h_d = nc.declare_dram_parameter("targ_h", [BPC, 4, M + 3], f32,
                                         isOutput=False)
    pw_d = nc.declare_dram_parameter("pw", [BPC, KW], f32, isOutput=False)
    tw_d = nc.declare_dram_parameter("tw", [BPC, KW], f32, isOutput=False)
    paf_d = nc.declare_dram_parameter("paf", [BPC, 9], f32, isOutput=False)
    taf_d = nc.declare_dram_parameter("taf", [BPC, 9], f32, isOutput=False)
    ident_d = nc.declare_dram_parameter("ident16", [128, 128], bf16, isOutput=False)
    out_d = nc.declare_dram_parameter("partials", [1, 8], f32, isOutput=True)

    from contextlib import ExitStack

    with tile.TileContext(nc) as tc, ExitStack() as ctx:
        consts = ctx.enter_context(tc.tile_pool(name="consts", bufs=1))
        augs = ctx.enter_context(tc.tile_pool(name="augs", bufs=2))
        augs1 = ctx.enter_context(tc.tile_pool(name="augs1", bufs=2))
        accs = ctx.enter_context(tc.tile_pool(name="accs", bufs=2))
        colp = ctx.enter_context(tc.tile_pool(name="colp", bufs=1))
        wide = ctx.enter_context(tc.tile_pool(name="wide", bufs=2))
        rowp = ctx.enter_context(tc.tile_pool(name="rowp", bufs=2))
        small = ctx.enter_context(tc.tile_pool(name="small", bufs=4))
        d2p = ctx.enter_context(tc.tile_pool(name="d2p", bufs=3, space="PSUM"))
        auxp = ctx.enter_context(tc.tile_pool(name="auxp", bufs=2, space="PSUM"))

        ident16 = consts.tile([128, 128], bf16)
        nc.sync.dma_start(out=ident16, in_=ident_d[:, :])
        ones128 = consts.tile([128, 1], f32)
        nc.vector.memset(ones128, 1.0)
        ones16 = consts.tile([3, N], f16)
        nc.vector.memset(ones16, 1.0)
        quart16 = consts.tile([3, N], f16)
        nc.vector.memset(quart16, 0.25)
        out_sb = consts.tile([1, 8], f32)
        nc.vector.memset(out_sb, 0.0)

        def prep(b):
            # ---- augmented pred lhsT stack [9, N] fp16 ----
            # rows 0:3 = p, rows 3:6 = p^2, rows 6:9 = 1
            aug_p16 = augs.tile([9, N], f16, tag="aug_p")
            nc.gpsimd.dma_start(out=aug_p16[0:3, :], in_=pred_aug_d[b])
            sq_p16 = augs1.tile([3, N], f16, tag="sq_p")
            for c in range(4):
                s = slice(c * (N // 4), (c + 1) * (N // 4))
                nc.gpsimd.tensor_mul(sq_p16[:, s], aug_p16[0:3, s],
                                     aug_p16[0:3, s])
                nc.gpsimd.dma_start(out=aug_p16[3:6, s], in_=sq_p16[:, s])
            nc.gpsimd.dma_start(out=aug_p16[6:9, :], in_=quart16)

            # ---- target transform + augmented rhs stack [9, M] fp16 ----
            # rows 0:3 = -2*t', rows 3:6 = 1, rows 6:9 = t'^2
            targ_h = augs1.tile([4, M + 3], f32, tag="targ_h")
            nc.gpsimd.dma_start(out=targ_h, in_=targ_h_d[b])
            mtile = targ_h[:, M:M + 3]
            aug_t16 = augs.tile([9, M], f16, tag="aug_t")
            nc.gpsimd.dma_start(out=aug_t16[3:6, :], in_=ones16)
            sq_t16 = augs1.tile([3, M], f16, tag="sq_t")
            for c in range(NMC):
                cs = slice(c * MC, (c + 1) * MC)
                tf = auxp.tile([3, MC], f32, tag="aux")
                nc.tensor.matmul(tf, mtile, targ_h[:, cs],
                                 start=True, stop=True)
                nc.scalar.mul(aug_t16[0:3, cs], tf, -2.0)
            # rows 6:9 hold (-2t')^2 = 4*t'^2; the pred-side ones row is
            # 0.25 so the contraction contributes t'^2 exactly
            for c in range(4):
                s = slice(c * (M // 4), (c + 1) * (M // 4))
                nc.gpsimd.tensor_mul(sq_t16[:, s], aug_t16[0:3, s],
                                     aug_t16[0:3, s])
                nc.gpsimd.dma_start(out=aug_t16[6:9, s], in_=sq_t16[:, s])
            return aug_p16, aug_t16

        state = {}

        def main_tile(b, i):
            # processes the PAIR of n-tiles (i, i+1); i is even. Matmuls and
            # converts land in one shared wide buffer so the row-min folds
            # run once per pair at double width (strided 3D APs).
            aug_p16, aug_t16, colacc, rowmins = state[b][:4]
            d2w = wide.tile([128, 2 * M], bf16, tag="d2w")
            for t in range(2):
                lhsT = aug_p16[:, (i + t) * 128:(i + t + 1) * 128]
                for h in range(4):
                    ps = d2p.tile([128, 1024], f32, tag="d2")
                    for q in range(2):
                        j = 2 * h + q
                        nc.tensor.matmul(ps[:, q * MC:(q + 1) * MC], lhsT,
                                         aug_t16[:, j * MC:(j + 1) * MC],
                                         start=True, stop=True)
                    nc.scalar.activation(
                        d2w[:, t * M + h * 1024:t * M + (h + 1) * 1024], ps,
                        Act.Relu)
                # col-min accumulate (bf16 2x mode), one op per tile
                half = d2w[:, t * M:(t + 1) * M]
                if i + t == 0:
                    nc.vector.tensor_copy(colacc, half)
                else:
                    nc.vector.tensor_tensor(colacc, half, colacc, Alu.min)
            # row mins for both tiles at once: 3 strided double-width folds
            dv = d2w.rearrange("p (t f) -> p t f", t=2)
            rsc = rowp.tile([128, 2, 2048], bf16, tag="rsc")
            nc.vector.tensor_tensor(rsc, dv[:, :, 0:2048],
                                    dv[:, :, 2048:4096], Alu.min)
            nc.vector.tensor_tensor(rsc[:, :, 0:1024], rsc[:, :, 0:1024],
                                    rsc[:, :, 1024:2048], Alu.min)
            k = i % 8
            if k == 0:
                rsc8_new = rowp.tile([128, 4096], bf16, tag="rsc8")
                state[b] = state[b][:4] + (rsc8_new,)
            rsc8 = state[b][4]
            nc.vector.tensor_tensor(rsc8[:, k * 512:(k + 2) * 512],
                                    rsc[:, :, 0:512], rsc[:, :, 512:1024],
                                    Alu.min)
            if k == 6:
                # pre-fold each tile's 512 down to 64 with 2x-mode strided
                # TTs before the (1x-only) reduce, batched over 8 tiles
                r8 = rsc8.rearrange("p (a f) -> p a f", a=8)
                nc.vector.tensor_tensor(r8[:, :, 0:256], r8[:, :, 0:256],
                                        r8[:, :, 256:512], Alu.min)
                nc.vector.tensor_tensor(r8[:, :, 0:128], r8[:, :, 0:128],
                                        r8[:, :, 128:256], Alu.min)
                nc.vector.tensor_tensor(r8[:, :, 0:64], r8[:, :, 0:64],
                                        r8[:, :, 64:128], Alu.min)
                nc.vector.tensor_reduce(rowmins[:, i - 6:i + 2],
                                        r8[:, :, 0:64], Ax.X, Alu.min)

        def start_batch(b):
            aug_p16, aug_t16 = prep(b)
            colacc = colp.tile([128, M], bf16, tag=f"colacc{b}")
            rowmins = accs.tile([128, NT], f32, tag="rowmins")
            state[b] = (aug_p16, aug_t16, colacc, rowmins)

        def end_batch(b):
            _, _, colacc, rowmins = state[b][:4]
            # row side: sum, partition-sum via ones matmul
            rsum = small.tile([128, 1], f32, tag="rsum")
            nc.vector.tensor_reduce(rsum, rowmins, Ax.X, Alu.add)
            ssc = auxp.tile([1, 1], f32, tag="aux")
            nc.tensor.matmul(ssc, rsum, ones128, start=True, stop=True)
            nc.scalar.copy(out_sb[0:1, 2 * b:2 * b + 1], ssc)
            # col side: transpose 128-wide chunks, reduce over free
            colmins = accs.tile([128, NT], f32, tag="colmins")
            for g in range(NT // 8):
                tp = auxp.tile([128, 1024], bf16, tag="aux")
                for k in range(8):
                    j = 8 * g + k
                    nc.tensor.transpose(tp[:, k * 128:(k + 1) * 128],
                                        colacc[:, j * 128:(j + 1) * 128],
                                        ident16)
                nc.vector.tensor_reduce(
                    colmins[:, 8 * g:8 * g + 8],
                    tp.rearrange("p (a f) -> p a f", a=8), Ax.X, Alu.min)
            csum = small.tile([128, 1], f32, tag="rsum")
            nc.vector.tensor_reduce(csum, colmins, Ax.X, Alu.add)
            ssc = auxp.tile([1, 1], f32, tag="aux")
            nc.tensor.matmul(ssc, csum, ones128, start=True, stop=True)
            nc.scalar.copy(out_sb[0:1, 2 * b + 1:2 * b + 2], ssc)

        # coeff + affine MSE partial sums (no deps on the big pipeline; they
        # fill the cold-start window)
        wdif = small.tile([BPC, KW], f32, tag="wdif")
        twt = small.tile([BPC, KW], f32, tag="twt")
        nc.sync.dma_start(out=wdif, in_=pw_d[:, :])
        nc.sync.dma_start(out=twt, in_=tw_d[:, :])
        nc.vector.tensor_sub(wdif, wdif, twt)
        nc.scalar.activation(wdif, wdif, Act.Square)
        wsum = small.tile([BPC, 1], f32, tag="wsum")
        nc.vector.tensor_reduce(wsum, wdif, Ax.X, Alu.add)
        ssc = auxp.tile([1, 1], f32, tag="aux")
        nc.tensor.matmul(ssc, wsum, ones128[0:BPC, :], start=True, stop=True)
        nc.scalar.copy(out_sb[0:1, 4:5], ssc)

        adif = small.tile([BPC, 9], f32, tag="adif")
        taft = small.tile([BPC, 9], f32, tag="taft")
        nc.sync.dma_start(out=adif, in_=paf_d[:, :])
        nc.sync.dma_start(out=taft, in_=taf_d[:, :])
        nc.vector.tensor_sub(adif, adif, taft)
        nc.scalar.activation(adif, adif, Act.Square)
        asum = small.tile([BPC, 1], f32, tag="asum")
        nc.vector.tensor_reduce(asum, adif, Ax.X, Alu.add)
        ssc = auxp.tile([1, 1], f32, tag="aux")
        nc.tensor.matmul(ssc, asum, ones128[0:BPC, :], start=True, stop=True)
        nc.scalar.copy(out_sb[0:1, 5:6], ssc)

        # software-pipelined schedule: batch b+1's prep and first tiles are
        # emitted before batch b's tail reduction so no engine drains dry at
        # the boundary
        start_batch(0)
        for i in range(0, NT - 4, 2):
            main_tile(0, i)
        start_batch(1)
        for i in range(NT - 4, NT, 2):
            main_tile(0, i)
        for i in range(0, 6, 2):
            main_tile(1, i)
        end_batch(0)
        for i in range(6, NT, 2):
            main_tile(1, i)
        end_batch(1)

        # ---- write out ----
        nc.sync.dma_start(out=out_d[:, :], in_=out_sb)

    nc.finalize()
    return nc


def _euler_xyz_to_matrix(angles):
    ax, ay, az = angles[:, 0], angles[:, 1], angles[:, 2]
    cx, sx = np.cos(ax), np.sin(ax)
    cy, sy = np.cos(ay), np.sin(ay)
    cz, sz = np.cos(az), np.sin(az)
    one = np.ones_like(cx)
    zero = np.zeros_like(cx)
    Rx = np.stack([one, zero, zero,
                   zero, cx, -sx,
                   zero, sx, cx], axis=-1).reshape(-1, 3, 3)
    Ry = np.stack([cy, zero, sy,
                   zero, one, zero,
                   -sy, zero, cy], axis=-1).reshape(-1, 3, 3)
    Rz = np.stack([cz, -sz, zero,
                   sz, cz, zero,
                   zero, zero, one], axis=-1).reshape(-1, 3, 3)
    return Rx @ Ry @ Rz


def kernel(pred_shape, pred_weights, pred_affine, targ_shape, targ_weights,
           targ_affine):
    import ml_dtypes
    from concourse.bass_utils import run_bass_kernel_spmd

    if "nc" not in _cache:
        _cache["nc"] = _build_nc()
    nc = _cache["nc"]

    # accept numpy or jax arrays
    pred_shape = np.asarray(pred_shape)
    pred_weights = np.asarray(pred_weights)
    pred_affine = np.asarray(pred_affine)
    targ_shape = np.asarray(targ_shape)
    targ_weights = np.asarray(targ_weights)
    targ_affine = np.asarray(targ_affine)

    f32 = np.float32
    rot = targ_affine[:, 0:3].astype(np.float64)
    trans = targ_affine[:, 3:6].astype(np.float64)
    scale = targ_affine[:, 6:9].astype(np.float64)
    R = _euler_xyz_to_matrix(rot)  # [B,3,3]

    # transform lhsT rows: out[e,m] = sum_d mrows[d,e] * targ_h[d,m] = t'_e[m]
    mrows = np.zeros((B, 4, 3), dtype=f32)
    for e in range(3):
        mrows[:, 0:3, e] = (R[:, :, e] * scale[:, e:e + 1]).astype(f32)
        mrows[:, 3, e] = trans[:, e].astype(f32)

    predT = np.ascontiguousarray(pred_shape.transpose(0, 2, 1), dtype=f32)
    targT = np.ascontiguousarray(targ_shape.transpose(0, 2, 1), dtype=f32)

    targ_h = np.empty((B, 4, M + 3), dtype=f32)
    targ_h[:, 0:3, 0:M] = targT
    targ_h[:, 3, 0:M] = 1.0
    targ_h[:, :, M:M + 3] = mrows

    ident16 = np.eye(128, dtype=ml_dtypes.bfloat16)

    in_maps = []
    for c in range(NCORES):
        s = slice(c * BPC, (c + 1) * BPC)
        in_maps.append({
            "pred_aug": predT[s],
            "targ_h": targ_h[s],
            "pw": np.ascontiguousarray(pred_weights[s], dtype=f32),
            "tw": np.ascontiguousarray(targ_weights[s], dtype=f32),
            "paf": np.ascontiguousarray(pred_affine[s], dtype=f32),
            "taf": np.ascontiguousarray(targ_affine[s], dtype=f32),
            "ident16": ident16,
        })

    _cache["in_maps"] = in_maps
    res = run_bass_kernel_spmd(nc, in_maps, list(range(NCORES))).results

    point_sum = 0.0
    coeff_sum = 0.0
    affine_sum = 0.0
    for c in range(NCORES):
        p = res[c]["partials"].reshape(-1)
        point_sum += float(p[0] + p[1] + p[2] + p[3])
        coeff_sum += float(p[4])
        affine_sum += float(p[5])

    point = point_sum / (B * N)
    coeff = coeff_sum / (B * KW)
    affine = affine_sum / (B * 9)
    total = W_POINT * point + W_COEFF * coeff + W_AFFINE * affine
    return (np.float32(total), np.float32(point), np.float32(coeff),
            np.float32(affine))

